# revision 1
# baseline (speedup 1.0000x reference)
"""Self-contained Trainium2 Bass kernel for single-head full-dim attention.

Reference computation (fp32 jax):
    q  = x @ Wq                      # [B, Nq, D]
    kv = y @ Wkv                     # [B, Nkv, 2D] -> k, v
    attn = softmax(q * D^-0.5 @ k^T) # [B, Nq, Nkv]
    out  = attn @ v                  # [B, Nq, D]
with B=4, Nq=Nkv=2048, D=1024.

Distribution: data parallel over 8 NeuronCores, shard = (batch b,
kv-half s).  Each core computes q for ALL 2048 queries of its batch
(cheap, duplicated across the pair), K/V for its 1024 keys, the
2048x1024 exp-score block, and the UNNORMALIZED output block
out'_s = exp(S_s) @ v_s plus the partial softmax denominator
Z_s = sum_k exp(S_s).  The host combines the two halves:
out = (out'_0 + out'_1) / (Z_0 + Z_1).  This avoids both collectives
and the (2x more expensive) duplicated K/V compute of a query-sharded
layout.

Layout trick: everything on-chip is computed transposed
([feature, token]) so the TensorEngine can contract along partitions
without any on-chip transposes.  The host pre-transposes x and y, folds
the D^-0.5 scale into Wq, and converts all matmul operands to bf16
(fp32 PSUM accumulation).  Softmax uses exp without max-subtraction
(scores ~ N(0,1) by construction; fp32 exp is safe) on the scalar
engine; Z is a ones-vector matmul.
"""

import numpy as np
import ml_dtypes

import concourse.bass as bass
import concourse.mybir as mybir
import concourse.tile as tile
from concourse.bass import ds
from concourse.bass_utils import run_bass_kernel_spmd

DIM = 1024
B = 4
NQ = 2048
NKV = 2048
N_CORES = 8
NKV_SHARD = 1024  # keys per core

BF16 = mybir.dt.bfloat16
F32 = mybir.dt.float32
NP_BF16 = ml_dtypes.bfloat16


def _split_sync_waits(nc, max_waits: int = 1):
    """walrus in this toolchain rejects instructions carrying more than one
    sem wait ("Too many sync wait commands").  Hoist extra waits onto
    preceding same-engine NOPs: the engine dispatches in order, so waiting
    just before the instruction is semantically identical (at worst it
    delays issue slightly)."""
    import bass_rust as _bass_rust

    for f in nc.m.functions:
        for bb in f.blocks:
            insts = list(bb.instructions)
            out = []
            changed = False
            for inst in insts:
                si = getattr(inst, "sync_info", None)
                waits = list(si.on_wait) if si is not None and si.on_wait else []
                if len(waits) > max_waits:
                    changed = True
                    extra, keep = waits[:-max_waits], waits[-max_waits:]
                    for k in range(0, len(extra), max_waits):
                        nop = mybir.InstNoOp(
                            name=f"{inst.name}_sw{k}", engine=inst.engine,
                            ins=[], outs=[],
                        )
                        nop.sync_info = _bass_rust.SyncInfo(
                            on_wait=extra[k : k + max_waits], on_update=[]
                        )
                        out.append(nop)
                    si.on_wait = keep
                    inst.sync_info = si
                out.append(inst)
            if changed:
                bb.instructions = out


def build_attention_nc():
    """Build the per-core Bass graph (identical on all 8 cores)."""
    nc = bass.Bass()

    # DRAM parameters (per-core shards, host-prepped layouts; all bf16
    # except the f32 outputs).
    xT_d = nc.declare_dram_parameter("xT", [DIM, NQ], BF16, isOutput=False)
    yT_d = nc.declare_dram_parameter("yT", [DIM, NKV_SHARD], BF16, isOutput=False)
    # wq/wk: column slabs: [do_chunk, d_in, 128], slab j = W[:, j*128:(j+1)*128]
    wq_d = nc.declare_dram_parameter("wq", [8, DIM, 128], BF16, isOutput=False)
    wk_d = nc.declare_dram_parameter("wk", [8, DIM, 128], BF16, isOutput=False)
    wv_d = nc.declare_dram_parameter("wv", [DIM, DIM], BF16, isOutput=False)
    out_d = nc.declare_dram_parameter("out", [NQ, DIM], F32, isOutput=True)
    # Z output in column-major tile layout: z[t*128 + p] = zout[p, t]
    z_d = nc.declare_dram_parameter("zout", [128, 16], F32, isOutput=True)

    with tile.TileContext(nc) as tc:
        # Long-lived pool: on-chip intermediates live to the end.
        L = tc.alloc_tile_pool(name="L", bufs=1)
        pm = tc.alloc_tile_pool(name="pm", bufs=1, space="PSUM")
        # Transient input pools, released once consumed (LIFO: t2 first).
        t1 = tc.alloc_tile_pool(name="t1", bufs=1)
        t2 = tc.alloc_tile_pool(name="t2", bufs=1)

        # ---- HAM warm-up: ~24 dummy matmuls on a zeroed scratch tile run
        # during the otherwise-idle input-DMA window, flipping the PE clock
        # gate to 8/8 (2.4GHz) before the first real matmul arrives.
        ws = t1.tile([128, 512], BF16, name="warm", tag="warm", bufs=1)
        nc.vector.memset(ws[:], 0.0)
        wps = pm.tile([128, 512], F32, name="wps", tag="z", bufs=2)
        for w in range(24):
            nc.tensor.matmul(
                wps[:], lhsT=ws[:, 0:128], rhs=ws[:],
                start=(w == 0), stop=(w == 23),
            )

        # ---- P2 first: its inputs (yt 2MB + wk slab 256KB) are the
        # smallest, so the PE starts ~6us in; xt (4MB) + wv land in the
        # background during P2/P3.
        # yt split into per-chunk tiles/DMAs: the first P2 matmul only
        # gates on wk slab 0 + yt chunk 0 (512KB), not the whole input set.
        ytr = yT_d.rearrange("(c p) n -> c p n", p=128)
        kt = [L.tile([128, NKV_SHARD], BF16, name=f"kt{j}", tag="kt", bufs=8) for j in range(8)]
        wk_slabs = []
        ytc = []
        for c in range(8):
            slab = t2.tile([128, 8, 128], BF16, name=f"wk{c}", tag="wk", bufs=8)
            nc.sync.dma_start(
                out=slab[:], in_=wk_d[c].rearrange("(c p) m -> p c m", p=128)
            )
            wk_slabs.append(slab)
            t = t2.tile([128, NKV_SHARD], BF16, name=f"yt{c}", tag="yt", bufs=8)
            nc.sync.dma_start(out=t[:], in_=ytr[c])
            ytc.append(t)
        wv = t2.tile([128, 8, DIM], BF16, name="wv", bufs=1)
        nc.sync.dma_start(out=wv[:], in_=wv_d.rearrange("(c p) n -> p c n", p=128))
        xt = t1.tile([128, 8, NQ], BF16, name="xt", bufs=1)
        nc.sync.dma_start(out=xt[:], in_=xT_d.rearrange("(c p) n -> p c n", p=128))

        for j in range(8):
            slab = wk_slabs[j]
            for q in range(2):  # nkv 512-chunk
                ps = pm.tile([128, 512], F32, name=f"psk{j}_{q}", tag="mm", bufs=4)
                for c in range(8):
                    nc.tensor.matmul(
                        ps[:],
                        lhsT=slab[:, c, :],
                        rhs=ytc[c][:, ds(q * 512, 512)],
                        start=(c == 0),
                        stop=(c == 7),
                    )
                nc.any.tensor_copy(kt[j][:, ds(q * 512, 512)], ps[:])

        # ---- P3: v[nkv, do] = sum_di yT[di, nkv] * Wv[di, do] -----------
        vt = [L.tile([128, DIM], BF16, name=f"v{i}", tag="v", bufs=8) for i in range(8)]
        for i in range(8):  # nkv 128-tile
            for d in range(2):  # d_out 512-chunk
                ps = pm.tile([128, 512], F32, name=f"psv{i}_{d}", tag="mm", bufs=4)
                for c in range(8):
                    nc.tensor.matmul(
                        ps[:],
                        lhsT=ytc[c][:, ds(i * 128, 128)],
                        rhs=wv[:, c, ds(d * 512, 512)],
                        start=(c == 0),
                        stop=(c == 7),
                    )
                nc.any.tensor_copy(vt[i][:, ds(d * 512, 512)], ps[:])
        t2.release()

        # ---- P1: qT[do, nq] = sum_di Wq_s[di, do] * xT[di, nq] ----------
        qt = [L.tile([128, NQ], BF16, name=f"qt{j}", tag="qt", bufs=8) for j in range(8)]
        for j in range(8):  # d_out chunk
            slab = t1.tile([128, 8, 128], BF16, name=f"wq{j}", tag="wq", bufs=3)
            nc.sync.dma_start(
                out=slab[:], in_=wq_d[j].rearrange("(c p) m -> p c m", p=128)
            )
            for q in range(4):  # nq 512-chunk
                ps = pm.tile([128, 512], F32, name=f"psq{j}_{q}", tag="mm", bufs=4)
                for c in range(8):  # d_in chunk (contraction)
                    nc.tensor.matmul(
                        ps[:],
                        lhsT=slab[:, c, :],
                        rhs=xt[:, c, ds(q * 512, 512)],
                        start=(c == 0),
                        stop=(c == 7),
                    )
                nc.any.tensor_copy(qt[j][:, ds(q * 512, 512)], ps[:])
        t1.release()

        # ---- P4: expT[nkv, nq] = exp(sum_do kT[do,nkv] * qT[do,nq]) -----
        et = [L.tile([128, NQ], BF16, name=f"e{i}", tag="et", bufs=8) for i in range(8)]
        for i in range(8):  # nkv 128-tile
            for q in range(4):  # nq 512-chunk
                ps = pm.tile([128, 512], F32, name=f"pse{i}_{q}", tag="mm", bufs=4)
                for j in range(8):  # d_out chunk (contraction)
                    nc.tensor.matmul(
                        ps[:],
                        lhsT=kt[j][:, ds(i * 128, 128)],
                        rhs=qt[j][:, ds(q * 512, 512)],
                        start=(j == 0),
                        stop=(j == 7),
                    )
                nc.scalar.activation(
                    et[i][:, ds(q * 512, 512)],
                    ps[:],
                    mybir.ActivationFunctionType.Exp,
                )

        # ---- P5: Z[nq] = sum_nkv expT[nkv, nq] ---------------------------
        ones = L.tile([128, 1], F32, name="ones", bufs=1)
        nc.vector.memset(ones[:], 1.0)
        one_f32 = L.tile([1, 1], F32, name="one_f32", bufs=1)
        nc.vector.memset(one_f32[:], 1.0)
        # Partial partition-sums on the (otherwise idle) vector engine: a
        # 3-level f32 add-tree collapses the 8 et tiles to one, so the PE
        # only streams 4 ones-matmuls instead of 32.
        t3 = tc.alloc_tile_pool(name="t3", bufs=1)
        s0 = [t3.tile([128, NQ], F32, name=f"es0_{h}", tag="es", bufs=3) for h in range(2)]
        nc.vector.tensor_add(s0[0][:], et[0][:], et[1][:])
        nc.vector.tensor_add(s0[1][:], et[2][:], et[3][:])
        s1 = t3.tile([128, NQ], F32, name="es1", tag="es2", bufs=2)
        nc.vector.tensor_add(s1[:], s0[0][:], s0[1][:])
        s0b = [t3.tile([128, NQ], F32, name=f"es0b_{h}", tag="es", bufs=3) for h in range(2)]
        nc.vector.tensor_add(s0b[0][:], et[4][:], et[5][:])
        nc.vector.tensor_add(s0b[1][:], et[6][:], et[7][:])
        s2 = t3.tile([128, NQ], F32, name="es2", tag="es2", bufs=2)
        nc.vector.tensor_add(s2[:], s0b[0][:], s0b[1][:])
        stot = t3.tile([128, NQ], F32, name="estot", tag="es", bufs=3)
        nc.vector.tensor_add(stot[:], s1[:], s2[:])
        # Z lands as [1, 512] psum rows; transpose each 128-wide piece to a
        # [128, 1] psum column with a K=1 matmul (lhsT = row chunk, rhs = 1).
        zps = pm.tile([128, 16], F32, name="zps", tag="zt", bufs=1)
        for q in range(4):
            psz = pm.tile([1, 512], F32, name=f"psz{q}", tag="z", bufs=2)
            nc.tensor.matmul(
                psz[:],
                lhsT=ones[:],
                rhs=stot[:, ds(q * 512, 512)],
                start=True,
                stop=True,
            )
            zrow = L.tile([1, 512], F32, name=f"zrow{q}", tag="zrow", bufs=2)
            nc.any.tensor_copy(zrow[:], psz[:])
            for t in range(4):
                nc.tensor.matmul(
                    zps[:, ds(q * 4 + t, 1)],
                    lhsT=zrow[0:1, ds(t * 128, 128)],
                    rhs=one_f32[:],
                    start=True,
                    stop=True,
                )
        zcol = L.tile([128, 16], F32, name="zcol", bufs=1)
        nc.any.tensor_copy(zcol[:], zps[:])
        nc.sync.dma_start(out=z_d[:], in_=zcol[:])
        t3.release()

        # ---- P7: out'[nq, do] = sum_nkv expT[nkv,nq] * v[nkv,do] --------
        for t in range(16):  # nq 128-tile
            for d in range(2):  # d_out 512-chunk
                ps = pm.tile([128, 512], F32, name=f"pso{t}_{d}", tag="mm", bufs=4)
                for i in range(8):  # nkv contraction
                    nc.tensor.matmul(
                        ps[:],
                        lhsT=et[i][:, ds(t * 128, 128)],
                        rhs=vt[i][:, ds(d * 512, 512)],
                        start=(i == 0),
                        stop=(i == 7),
                    )
                ob = L.tile([128, 512], F32, name=f"o{t}_{d}", tag="o", bufs=3)
                nc.any.tensor_copy(ob[:], ps[:])
                nc.sync.dma_start(
                    out=out_d[ds(t * 128, 128), ds(d * 512, 512)], in_=ob[:]
                )
        pm.release()
        L.release()

    _split_sync_waits(nc)
    return nc


_NC_CACHE = {}


def _get_nc():
    if "nc" not in _NC_CACHE:
        _NC_CACHE["nc"] = build_attention_nc()
    return _NC_CACHE["nc"]


def make_in_maps(x, y, Wq, Wkv):
    """Host-side sharding + layout prep. Returns in_maps for cores 0-7."""
    scale = DIM ** (-0.5)
    wq_s = (np.asarray(Wq, np.float32) * scale).astype(NP_BF16)
    wkv = np.asarray(Wkv, np.float32)
    wk = wkv[:, :DIM].astype(NP_BF16)
    wv = wkv[:, DIM:].astype(NP_BF16)
    # column slabs [8, DIM, 128]
    wq_slabs = np.ascontiguousarray(wq_s.reshape(DIM, 8, 128).transpose(1, 0, 2))
    wk_slabs = np.ascontiguousarray(wk.reshape(DIM, 8, 128).transpose(1, 0, 2))

    x = np.asarray(x, np.float32)
    y = np.asarray(y, np.float32)
    in_maps = []
    for core in range(N_CORES):
        b, s = divmod(core, 2)
        xT = np.ascontiguousarray(x[b].T).astype(NP_BF16)
        yT = np.ascontiguousarray(
            y[b, s * NKV_SHARD : (s + 1) * NKV_SHARD, :].T
        ).astype(NP_BF16)
        in_maps.append(
            {"xT": xT, "yT": yT, "wq": wq_slabs, "wk": wk_slabs, "wv": wv}
        )
    return in_maps


def run_sharded(x, y, Wq, Wkv, trace=False, tmpdir=None):
    """Run the SPMD kernel; returns (full_output, BassKernelResults)."""
    nc = _get_nc()
    in_maps = make_in_maps(x, y, Wq, Wkv)
    try:
        res = run_bass_kernel_spmd(
            nc, in_maps, core_ids=list(range(N_CORES)), trace=trace, tmpdir=tmpdir
        )
    except Exception:
        # one retry: transient NRT device states (e.g. a previous crashed
        # load) usually clear on the next attempt
        res = run_bass_kernel_spmd(
            nc, in_maps, core_ids=list(range(N_CORES)), trace=trace, tmpdir=tmpdir
        )
    out = np.empty((B, NQ, DIM), np.float32)
    for b in range(B):
        r0, r1 = res.results[2 * b], res.results[2 * b + 1]
        num = r0["out"] + r1["out"]
        z = (r0["zout"] + r1["zout"]).T.reshape(NQ)
        out[b] = num / z[:, None]
    return out, res


def kernel(x, y, Wq, Wkv):
    out, _ = run_sharded(x, y, Wq, Wkv)
    return out



# revision 2
# speedup vs baseline: 1.1819x; 1.1819x over previous
"""Self-contained Trainium2 Bass kernel for single-head full-dim attention.

Reference computation (fp32 jax):
    q  = x @ Wq                      # [B, Nq, D]
    kv = y @ Wkv                     # [B, Nkv, 2D] -> k, v
    attn = softmax(q * D^-0.5 @ k^T) # [B, Nq, Nkv]
    out  = attn @ v                  # [B, Nq, D]
with B=4, Nq=Nkv=2048, D=1024.

Distribution: data parallel over 8 NeuronCores, shard = (batch b,
kv-half s).  Each core handles all 2048 queries of its batch against
its 1024 keys, producing the UNNORMALIZED output block
out'_s = exp(S_s) @ v_s and the partial softmax denominator
Z_s = sum_k exp(S_s).  The host combines the two halves:
out = (out'_0 + out'_1) / (Z_0 + Z_1).  No collectives.

Key algebraic trick: the Q and K projections are folded into a single
host-precomputed matrix M = (Wq * D^-0.5) @ Wk^T, so
    scores = (x @ M) @ y^T.
This removes the K projection entirely and de-duplicates the Q
projection across the core pair "for free": per-core flops hit the
ideal total/8 (15.0 GF vs 17.2 GF for the q/k-projection layout).

Layouts: everything on-chip is computed transposed ([feature, token])
so the TensorEngine contracts along partitions without any on-chip
transposes; the unnormalized output is produced transposed
([d_out, nq]) which lets the stationary operand be a v-tile and the
moving operand stream full 512-wide nq chunks.  All matmul operands
bf16 (fp32 PSUM accumulation); the out' result is stored bf16 (the
2e-2 rel tolerance dwarfs bf16 rounding).  exp runs without
max-subtraction (scores ~ N(0,1) by construction) on the scalar
engine; Z is a ones-vector matmul emitted after P7 so its serial
chain hides behind the output-DMA drain.
"""

import numpy as np
import ml_dtypes

import concourse.bass as bass
import concourse.mybir as mybir
import concourse.tile as tile
from concourse.bass import ds
from concourse.bass_utils import run_bass_kernel_spmd

DIM = 1024
B = 4
NQ = 2048
NKV = 2048
N_CORES = 8
NKV_SHARD = 1024  # keys per core

BF16 = mybir.dt.bfloat16
F32 = mybir.dt.float32
NP_BF16 = ml_dtypes.bfloat16


def _split_sync_waits(nc, max_waits: int = 1):
    """walrus in this toolchain rejects instructions carrying more than one
    sem wait ("Too many sync wait commands").  Hoist extra waits onto
    preceding same-engine NOPs: the engine dispatches in order, so waiting
    just before the instruction is semantically identical (at worst it
    delays issue slightly)."""
    import bass_rust as _bass_rust

    for f in nc.m.functions:
        for bb in f.blocks:
            insts = list(bb.instructions)
            out = []
            changed = False
            for inst in insts:
                si = getattr(inst, "sync_info", None)
                waits = list(si.on_wait) if si is not None and si.on_wait else []
                if len(waits) > max_waits:
                    changed = True
                    extra, keep = waits[:-max_waits], waits[-max_waits:]
                    for k in range(0, len(extra), max_waits):
                        nop = mybir.InstNoOp(
                            name=f"{inst.name}_sw{k}", engine=inst.engine,
                            ins=[], outs=[],
                        )
                        nop.sync_info = _bass_rust.SyncInfo(
                            on_wait=extra[k : k + max_waits], on_update=[]
                        )
                        out.append(nop)
                    si.on_wait = keep
                    inst.sync_info = si
                out.append(inst)
            if changed:
                bb.instructions = out


def build_attention_nc():
    """Build the per-core Bass graph (identical on all 8 cores)."""
    nc = bass.Bass()

    # DRAM parameters (per-core shards, host-prepped layouts; all bf16
    # except the f32 z output).
    xT_d = nc.declare_dram_parameter("xT", [DIM, NQ], BF16, isOutput=False)
    yT_d = nc.declare_dram_parameter("yT", [DIM, NKV_SHARD], BF16, isOutput=False)
    # m: column slabs of M = Wq_scaled @ Wk^T: [do_chunk, d_in, 128],
    # slab j = M[:, j*128:(j+1)*128]
    m_d = nc.declare_dram_parameter("m", [8, DIM, 128], BF16, isOutput=False)
    wv_d = nc.declare_dram_parameter("wv", [DIM, DIM], BF16, isOutput=False)
    # transposed unnormalized output out'^T [d_out, nq], bf16
    outT_d = nc.declare_dram_parameter("outT", [DIM, NQ], BF16, isOutput=True)
    z_d = nc.declare_dram_parameter("zout", [1, NQ], F32, isOutput=True)

    with tile.TileContext(nc) as tc:
        # Long-lived pool: on-chip intermediates live to the end.
        L = tc.alloc_tile_pool(name="L", bufs=1)
        pm = tc.alloc_tile_pool(name="pm", bufs=1, space="PSUM")
        # Transient pools, released once consumed (LIFO release order).
        tx = tc.alloc_tile_pool(name="tx", bufs=1)  # xt + m slabs
        tw = tc.alloc_tile_pool(name="tw", bufs=1)  # wv

        # ---- HAM warm-up: ~24 dummy matmuls on a zeroed scratch tile run
        # during the otherwise-idle input-DMA window, flipping the PE clock
        # gate to full speed before the first real matmul arrives.
        ws = L.tile([128, 512], BF16, name="warm", bufs=1)
        nc.vector.memset(ws[:], 0.0)
        wps = pm.tile([128, 512], F32, name="wps", tag="mm", bufs=7)
        for w in range(24):
            nc.tensor.matmul(
                wps[:], lhsT=ws[:, 0:128], rhs=ws[:],
                start=(w == 0), stop=(w == 23),
            )

        # ---- Input DMAs, priority order.  P3 (V projection) runs first and
        # needs yT + Wv; its c-th contraction step gates only on chunk c.
        # P1 (x@M) needs m slab j + ALL of xT, so m[0] is issued before xT
        # and the remaining slabs after (they land during P3/P1).
        ytr = yT_d.rearrange("(c p) n -> c p n", p=128)
        wvr = wv_d.rearrange("(c p) n -> p c n", p=128)
        wv = tw.tile([128, 8, DIM], BF16, name="wv", bufs=1)
        ytc = []
        for c in range(8):
            t = L.tile([128, NKV_SHARD], BF16, name=f"yt{c}", tag="yt", bufs=8)
            nc.sync.dma_start(out=t[:], in_=ytr[c])
            ytc.append(t)
            nc.sync.dma_start(out=wv[:, c, :], in_=wvr[:, c, :])
        msl = [tx.tile([128, 8, 128], BF16, name=f"m{j}", tag="m", bufs=8)
               for j in range(8)]
        nc.sync.dma_start(
            out=msl[0][:], in_=m_d[0].rearrange("(c p) m -> p c m", p=128)
        )
        xt = tx.tile([128, 8, NQ], BF16, name="xt", bufs=1)
        nc.sync.dma_start(out=xt[:], in_=xT_d.rearrange("(c p) n -> p c n", p=128))
        for j in range(1, 8):
            nc.sync.dma_start(
                out=msl[j][:], in_=m_d[j].rearrange("(c p) m -> p c m", p=128)
            )

        # ---- P3: v[nkv, do] = sum_c yT[c-chunk, nkv].T @ Wv[c-chunk, do]
        vt = [L.tile([128, DIM], BF16, name=f"v{i}", tag="v", bufs=8) for i in range(8)]
        for i in range(8):  # nkv 128-tile
            ps = [pm.tile([128, 512], F32, name=f"psv{i}_{h}", tag="mm", bufs=7)
                  for h in range(2)]
            for c in range(8):  # d_in chunk (contraction)
                for h in range(2):  # d_out 512-chunk
                    nc.tensor.matmul(
                        ps[h][:],
                        lhsT=ytc[c][:, ds(i * 128, 128)],
                        rhs=wv[:, c, ds(h * 512, 512)],
                        start=(c == 0),
                        stop=(c == 7),
                    )
            for h in range(2):
                nc.any.tensor_copy(vt[i][:, ds(h * 512, 512)], ps[h][:])
        tw.release()

        # ---- P1: tT[dm, nq] = sum_c M[c-chunk, dm-slab].T @ xT[c-chunk, nq]
        tt = [L.tile([128, NQ], BF16, name=f"t{j}", tag="tt", bufs=8) for j in range(8)]
        for j in range(8):  # dm slab
            ps = [pm.tile([128, 512], F32, name=f"pst{j}_{q}", tag="mm", bufs=7)
                  for q in range(4)]
            for c in range(8):  # d_in chunk (contraction)
                for q in range(4):  # nq 512-chunk
                    nc.tensor.matmul(
                        ps[q][:],
                        lhsT=msl[j][:, c, :],
                        rhs=xt[:, c, ds(q * 512, 512)],
                        start=(c == 0),
                        stop=(c == 7),
                    )
            for q in range(4):
                nc.any.tensor_copy(tt[j][:, ds(q * 512, 512)], ps[q][:])
        tx.release()

        # ---- P4: expT[nkv, nq] = exp(sum_c yT[c,nkv].T @ tT[c,nq]) --------
        et = [L.tile([128, NQ], BF16, name=f"e{i}", tag="et", bufs=8) for i in range(8)]
        for i in range(8):  # nkv 128-tile
            ps = [pm.tile([128, 512], F32, name=f"pse{i}_{q}", tag="mm", bufs=7)
                  for q in range(4)]
            for c in range(8):  # dm chunk (contraction)
                for q in range(4):  # nq 512-chunk
                    nc.tensor.matmul(
                        ps[q][:],
                        lhsT=ytc[c][:, ds(i * 128, 128)],
                        rhs=tt[c][:, ds(q * 512, 512)],
                        start=(c == 0),
                        stop=(c == 7),
                    )
            for q in range(4):
                nc.scalar.activation(
                    et[i][:, ds(q * 512, 512)],
                    ps[q][:],
                    mybir.ActivationFunctionType.Exp,
                )

        # ---- P7: out'^T[do, nq] = sum_i v[i-tile, do-slab].T @ expT[i, nq]
        for d in range(8):  # d_out 128-tile
            ps = [pm.tile([128, 512], F32, name=f"pso{d}_{q}", tag="mm", bufs=7)
                  for q in range(4)]
            for i in range(8):  # nkv contraction
                for q in range(4):  # nq 512-chunk
                    nc.tensor.matmul(
                        ps[q][:],
                        lhsT=vt[i][:, ds(d * 128, 128)],
                        rhs=et[i][:, ds(q * 512, 512)],
                        start=(i == 0),
                        stop=(i == 7),
                    )
            for q in range(4):
                ob = L.tile([128, 512], BF16, name=f"o{d}_{q}", tag="o", bufs=4)
                nc.any.tensor_copy(ob[:], ps[q][:])
                nc.sync.dma_start(
                    out=outT_d[ds(d * 128, 128), ds(q * 512, 512)], in_=ob[:]
                )

        # ---- P5: Z[nq] = sum_nkv expT[nkv, nq], emitted after P7 ---------
        # The vector add-tree depends only on et, so it runs during P4/P7;
        # the PE ones-matmuls + z DMA land in the output-drain window.
        ones = L.tile([128, 1], F32, name="ones", bufs=1)
        nc.vector.memset(ones[:], 1.0)
        tz = tc.alloc_tile_pool(name="tz", bufs=1)
        s0 = [tz.tile([128, NQ], F32, name=f"es0_{h}", tag="es", bufs=3) for h in range(2)]
        nc.vector.tensor_add(s0[0][:], et[0][:], et[1][:])
        nc.vector.tensor_add(s0[1][:], et[2][:], et[3][:])
        s1 = tz.tile([128, NQ], F32, name="es1", tag="es2", bufs=2)
        nc.vector.tensor_add(s1[:], s0[0][:], s0[1][:])
        s0b = [tz.tile([128, NQ], F32, name=f"es0b_{h}", tag="es", bufs=3) for h in range(2)]
        nc.vector.tensor_add(s0b[0][:], et[4][:], et[5][:])
        nc.vector.tensor_add(s0b[1][:], et[6][:], et[7][:])
        s2 = tz.tile([128, NQ], F32, name="es2", tag="es2", bufs=2)
        nc.vector.tensor_add(s2[:], s0b[0][:], s0b[1][:])
        stot = tz.tile([128, NQ], F32, name="estot", tag="es", bufs=3)
        nc.vector.tensor_add(stot[:], s1[:], s2[:])
        for q in range(4):
            psz = pm.tile([1, 512], F32, name=f"psz{q}", tag="z", bufs=1)
            nc.tensor.matmul(
                psz[:],
                lhsT=ones[:],
                rhs=stot[:, ds(q * 512, 512)],
                start=True,
                stop=True,
            )
            zrow = L.tile([1, 512], F32, name=f"zrow{q}", tag="zrow", bufs=2)
            nc.any.tensor_copy(zrow[:], psz[:])
            nc.sync.dma_start(out=z_d[0:1, ds(q * 512, 512)], in_=zrow[:])
        tz.release()
        pm.release()
        L.release()

    _split_sync_waits(nc)
    return nc


_NC_CACHE = {}


def _get_nc():
    if "nc" not in _NC_CACHE:
        _NC_CACHE["nc"] = build_attention_nc()
    return _NC_CACHE["nc"]


def make_in_maps(x, y, Wq, Wkv):
    """Host-side sharding + layout prep. Returns in_maps for cores 0-7."""
    scale = DIM ** (-0.5)
    wq_s = np.asarray(Wq, np.float32) * scale
    wkv = np.asarray(Wkv, np.float32)
    wk = wkv[:, :DIM]
    wv = wkv[:, DIM:].astype(NP_BF16)
    # M = Wq_scaled @ Wk^T, f32 accumulate then bf16; column slabs [8, DIM, 128]
    m = (wq_s @ wk.T).astype(NP_BF16)
    m_slabs = np.ascontiguousarray(m.reshape(DIM, 8, 128).transpose(1, 0, 2))

    x = np.asarray(x, np.float32)
    y = np.asarray(y, np.float32)
    in_maps = []
    for core in range(N_CORES):
        b, s = divmod(core, 2)
        xT = np.ascontiguousarray(x[b].T).astype(NP_BF16)
        yT = np.ascontiguousarray(
            y[b, s * NKV_SHARD : (s + 1) * NKV_SHARD, :].T
        ).astype(NP_BF16)
        in_maps.append({"xT": xT, "yT": yT, "m": m_slabs, "wv": wv})
    return in_maps


def run_sharded(x, y, Wq, Wkv, trace=False, tmpdir=None):
    """Run the SPMD kernel; returns (full_output, BassKernelResults)."""
    nc = _get_nc()
    in_maps = make_in_maps(x, y, Wq, Wkv)
    try:
        res = run_bass_kernel_spmd(
            nc, in_maps, core_ids=list(range(N_CORES)), trace=trace, tmpdir=tmpdir
        )
    except Exception:
        # one retry: transient NRT device states (e.g. a previous crashed
        # load) usually clear on the next attempt
        res = run_bass_kernel_spmd(
            nc, in_maps, core_ids=list(range(N_CORES)), trace=trace, tmpdir=tmpdir
        )
    out = np.empty((B, NQ, DIM), np.float32)
    for b in range(B):
        r0, r1 = res.results[2 * b], res.results[2 * b + 1]
        num = r0["outT"].astype(np.float32) + r1["outT"].astype(np.float32)
        z = (r0["zout"] + r1["zout"]).reshape(NQ)
        out[b] = (num / z[None, :]).T
    return out, res


def kernel(x, y, Wq, Wkv):
    out, _ = run_sharded(x, y, Wq, Wkv)
    return out
